# revision 13
# baseline (speedup 1.0000x reference)
"""Trainium2 Bass kernel for the AgentLoss problem.

Math: for each (l, b) the reference computes the masked cosine-similarity sum
    S = sum_{i != j} <x_i, x_j> / (|x_i| |x_j| + EPS)
over n=1024 agents with c=64 channels, then loss = sum_l mean_b S / (n(n-1)).

Since EPS (1e-5) is tiny vs |x_i||x_j| ~ 64, expand
    1/(m_i m_j + EPS) = r_i r_j - EPS r_i^2 r_j^2 + O(EPS^2),  r_i = 1/m_i
which makes the double sum separable:
    S ~= (|sum_i x_i r_i|^2 - sum_i msq_i r_i^2)
         - EPS * (|sum_i x_i r_i^2|^2 - sum_i msq_i r_i^4)
(order-1 truncation error ~3e-14 relative - validated vs fp64).

So the device only needs, per (l, b): row norms (square + segmented reduce),
reciprocal square roots, and thin matmuls contracting the agent axis with
[r, r^2] weight columns. Memory-bound as intended.

Sharding: data-parallel over batch b - core k takes b in {2k, 2k+1}, i.e.
8 (l, b_local) pairs per core. Each core emits a [2, 640] result block
(per-pair weighted column sums s, s2 and diag-correction partial sums);
the host does the final ~10k-flop combine in float64.
"""

import numpy as np

import concourse.bass as bass
import concourse.tile as tile
from concourse import bacc, mybir
from concourse.bass_utils import run_bass_kernel_spmd

EPS = 1e-5
L, B, N, C = 4, 16, 1024, 64
P = 128            # SBUF partitions
T = N // P         # 8 agent sub-rows per partition
NCORES = 8
BPC = B // NCORES  # b per core
NPAIR = L * BPC    # (l, b_local) pairs per core
GP = 4             # pairs per norm-batch group
NG = NPAIR // GP

F32 = mybir.dt.float32


def build_nc() -> bass.Bass:
    nc = bacc.Bacc(
        "TRN2", target_bir_lowering=False, debug=False, num_devices=NCORES
    )
    x = nc.declare_dram_parameter("x", [NPAIR, N, C], F32, isOutput=False)
    out = nc.declare_dram_parameter("out", [2, 640], F32, isOutput=True)

    with tile.TileContext(nc) as tc:
        with (
            tc.tile_pool(name="xpool", bufs=NPAIR) as xpool,
            tc.tile_pool(name="sqpool", bufs=NPAIR) as sqpool,
            tc.tile_pool(name="gpool", bufs=NG) as gpool,
            tc.tile_pool(name="cpool", bufs=1) as cpool,
            tc.tile_pool(name="pspool", bufs=1, space="PSUM") as pspool,
        ):
            ones = cpool.tile([P, 1], F32)
            nc.gpsimd.memset(ones[:], 1.0)
            stage = cpool.tile([2, 640], F32)
            nc.vector.memset(stage[:], 0.0)
            psum_s = pspool.tile([2, 512], F32)    # per pair j: cols 64j..64j+64, row0=s, row1=s2
            psum_pq = pspool.tile([1, 128], F32)   # per pair j: cols 16j..16j+16: [p-sums(8), q-sums(8)]
            psum_scr = pspool.tile([2, 2], F32)    # scratch: absorbs the RR wait (walrus allows 1 wait/matmul)

            for g in range(NG):
                xpairs = []
                msq = gpool.tile([P, 8 * GP], F32)
                for slot in range(GP):
                    j = g * GP + slot
                    xp = xpool.tile([P, T, C], F32)
                    nc.sync.dma_start(
                        out=xp[:], in_=x[j].rearrange("(p t) c -> p t c", p=P)
                    )
                    xsq = sqpool.tile([P, T, C], F32)
                    nc.scalar.square(xsq[:], xp[:])
                    nc.vector.tensor_reduce(
                        out=msq[:, slot * 8 : slot * 8 + 8],
                        in_=xsq[:],
                        axis=mybir.AxisListType.X,
                        op=mybir.AluOpType.add,
                    )
                    xpairs.append(xp)

                # r = rsqrt(msq) via reciprocal + ACT sqrt + one Newton step
                # (ACT sqrt LUT has a loose precision budget; NR squares the error)
                inv = gpool.tile([P, 8 * GP], F32)
                nc.vector.reciprocal(out=inv[:], in_=msq[:])
                r0 = gpool.tile([P, 8 * GP], F32)
                nc.scalar.sqrt(r0[:], inv[:])
                e = gpool.tile([P, 8 * GP], F32)
                nc.vector.tensor_mul(e[:], r0[:], r0[:])
                e2 = gpool.tile([P, 8 * GP], F32)
                nc.gpsimd.tensor_mul(e2[:], e[:], msq[:])
                cf = gpool.tile([P, 8 * GP], F32)
                nc.vector.tensor_scalar(
                    out=cf[:], in0=e2[:], scalar1=-0.5, scalar2=1.5,
                    op0=mybir.AluOpType.mult, op1=mybir.AluOpType.add,
                )
                RR = gpool.tile([P, 2 * 8 * GP], F32)  # cols 0:32 r, 32:64 r^2
                nc.vector.tensor_mul(RR[:, 0:32], r0[:], cf[:])
                nc.vector.tensor_mul(RR[:, 32:64], RR[:, 0:32], RR[:, 0:32])
                PQ = gpool.tile([P, 2 * 8 * GP], F32)  # cols 0:32 p=msq*r^2, 32:64 q=p*r^2
                nc.gpsimd.tensor_mul(PQ[:, 0:32], msq[:], RR[:, 32:64])
                nc.gpsimd.tensor_mul(PQ[:, 32:64], PQ[:, 0:32], RR[:, 32:64])

                nc.tensor.matmul(psum_scr[:], RR[:, 31::32], RR[:, 31::32], start=True, stop=True)

                for slot in range(GP):
                    j = g * GP + slot
                    xp = xpairs[slot]
                    for t in range(T):
                        idx = slot * 8 + t
                        nc.tensor.matmul(
                            psum_s[:, j * 64 : (j + 1) * 64],
                            RR[:, idx :: 32],          # [128, 2] = (r_t, r2_t)
                            xp[:, t, :],               # [128, 64]
                            start=(t == 0),
                            stop=(t == T - 1),
                        )
                    nc.tensor.matmul(
                        psum_pq[:, j * 16 : (j + 1) * 16],
                        ones[:],
                        PQ[:].rearrange("p (k s) -> p k s", k=2)[:, :, slot * 8 : slot * 8 + 8],
                        start=True,
                        stop=True,
                    )

            nc.vector.tensor_copy(stage[:, 0:512], psum_s[:])
            nc.vector.tensor_copy(stage[0:1, 512:640], psum_pq[:])
            nc.gpsimd.dma_start(out=out[:], in_=stage[:])

    nc.compile()
    return nc


_NC_CACHE = None


def _get_nc():
    global _NC_CACHE
    if _NC_CACHE is None:
        _NC_CACHE = build_nc()
    return _NC_CACHE


def run_cores(x_full: np.ndarray, trace: bool = False):
    """Shard, run on 8 NeuronCores, return (per-core out blocks, results obj)."""
    nc = _get_nc()
    in_maps = []
    for k in range(NCORES):
        shard = np.ascontiguousarray(
            x_full[:, BPC * k : BPC * (k + 1)].reshape(NPAIR, N, C)
        )
        in_maps.append({"x": shard})
    res = run_bass_kernel_spmd(nc, in_maps, list(range(NCORES)), trace=trace)
    outs = [res.results[k]["out"] for k in range(NCORES)]
    return outs, res


def reduce_host(outs) -> np.ndarray:
    total = 0.0
    for blk in outs:
        blk = blk.astype(np.float64)
        for j in range(NPAIR):
            s = blk[0, 64 * j : 64 * (j + 1)]
            s2 = blk[1, 64 * j : 64 * (j + 1)]
            pq = blk[0, 512 + 16 * j : 512 + 16 * (j + 1)]
            S0 = np.dot(s, s) - pq[0:8].sum()
            S1 = np.dot(s2, s2) - pq[8:16].sum()
            total += S0 - EPS * S1
    loss = total / (N * (N - 1)) / B
    return np.array(loss, dtype=np.float32)


def kernel(updated_agents: np.ndarray) -> np.ndarray:
    outs, _ = run_cores(np.asarray(updated_agents))
    return reduce_host(outs)


# revision 23
# speedup vs baseline: 1.0918x; 1.0918x over previous
"""Trainium2 Bass kernel for the AgentLoss problem (raw bacc, manual sems).

Math: for each (l, b) the reference computes the masked cosine-similarity sum
    S = sum_{i != j} <x_i, x_j> / (|x_i| |x_j| + EPS)
over n=1024 agents with c=64 channels, then loss = sum_l mean_b S / (n(n-1)).

Since EPS (1e-5) is tiny vs |x_i||x_j| ~ 64, expand
    1/(m_i m_j + EPS) = r_i r_j - EPS r_i^2 r_j^2 + O(EPS^2),  r_i = 1/m_i
which makes the double sum separable:
    S ~= (|sum_i x_i r_i|^2 - sum_i msq_i r_i^2)
         - EPS * (|sum_i x_i r_i^2|^2 - sum_i msq_i r_i^4)
(order-1 truncation error ~3e-14 relative - validated vs fp64).

Device work per (l, b) pair: row norms (ACT square + DVE segmented reduce),
rsqrt (DVE reciprocal + ACT sqrt + one Newton step), then thin fp32 matmuls
contracting the agent axis with [r, r^2] weight columns, packed two
sub-tiles per matmul (N=128, half-garbage outputs that the host discards).
Host does the final ~10k-flop combine in float64.

Sharding: data-parallel over batch b - core k takes b in {2k, 2k+1}, i.e.
8 (l, b_local) pairs per core. Each core returns a [4, 1152] block.
"""

from contextlib import ExitStack

import numpy as np

import concourse.bass as bass
from concourse import bacc, mybir
from concourse.bass_utils import run_bass_kernel_spmd

EPS = 1e-5
L, B, N, C = 4, 16, 1024, 64
P = 128            # SBUF partitions
T = N // P         # 8 agent sub-rows per partition
NCORES = 8
BPC = B // NCORES  # b per core
NPAIR = L * BPC    # (l, b_local) pairs per core
GP = 2             # pairs per norm-batch group
NG = NPAIR // GP   # 4 groups
GW = 8 * GP        # norm-group width (agents per partition per group)

F32 = mybir.dt.float32
OUT_W = NPAIR * P + NG * 2 * GW  # 1024 + 128


def build_nc() -> bass.Bass:
    nc = bacc.Bacc("TRN2", target_bir_lowering=False, debug=False, num_devices=NCORES)
    x = nc.declare_dram_parameter("x", [NPAIR, N, C], F32, isOutput=False)
    out = nc.declare_dram_parameter("out", [4, OUT_W], F32, isOutput=True)

    ctx = ExitStack()
    with ctx:
        def sb(name, shape):
            return ctx.enter_context(nc.sbuf_tensor(name, shape, F32))

        xp = [sb(f"xp{j}", [P, T, C]) for j in range(NPAIR)]
        xsq = [sb(f"xsq{j}", [P, T, C]) for j in range(NPAIR)]
        msq = [sb(f"msq{g}", [P, GW]) for g in range(NG)]
        inv = [sb(f"inv{g}", [P, GW]) for g in range(NG)]
        r0 = [sb(f"r0_{g}", [P, GW]) for g in range(NG)]
        e = [sb(f"e{g}", [P, GW]) for g in range(NG)]
        cf = [sb(f"cf{g}", [P, GW]) for g in range(NG)]
        RR = [sb(f"RR{g}", [P, 2 * GW]) for g in range(NG)]
        PQ = [sb(f"PQ{g}", [P, 2 * GW]) for g in range(NG)]
        ones = sb("ones", [P, 1])
        stage = sb("stage", [4, OUT_W])
        psum_s = ctx.enter_context(nc.psum_tensor("psum_s", [4, NPAIR * P], F32))
        psum_pq = ctx.enter_context(nc.psum_tensor("psum_pq", [1, NG * 2 * GW], F32))

        s_dma = [nc.alloc_semaphore(f"s_dma{j}") for j in range(NPAIR)]
        s_dmo = nc.alloc_semaphore("s_dmo")
        s_act = nc.alloc_semaphore("s_act")
        s_inv = nc.alloc_semaphore("s_inv")
        s_r0 = nc.alloc_semaphore("s_r0")
        s_rr = nc.alloc_semaphore("s_rr")
        s_pq = nc.alloc_semaphore("s_pq")
        s_pe = nc.alloc_semaphore("s_pe")
        s_stage = nc.alloc_semaphore("s_stage")
        s_dve = nc.alloc_semaphore("s_dve")   # DVE same-engine RAW chain
        s_gp = nc.alloc_semaphore("s_gp")     # GpSimd same-engine RAW chain
        sems = s_dma + [s_dmo, s_act, s_inv, s_r0, s_rr, s_pq, s_pe, s_stage,
                        s_dve, s_gp]

        with nc.Block() as block:

            @block.sync
            def _(sync):
                for j in range(NPAIR):
                    sync.dma_start(
                        out=xp[j][:], in_=x[j].rearrange("(p t) c -> p t c", p=P)
                    ).then_inc(s_dma[j], 16)
                sync.wait_ge(s_stage, 2)
                sync.dma_start(out=out[:], in_=stage[:]).then_inc(s_dmo, 16)
                sync.wait_ge(s_dmo, 16)

            @block.scalar
            def _(scalar):
                # squares run one group ahead of the sqrt for group g
                for g in range(NG):
                    for slot in range(GP):
                        j = g * GP + slot
                        scalar.square(xsq[j][:], xp[j][:])._wait_ge(
                            s_dma[j], 16
                        ).then_inc(s_act)
                    if g >= 1:
                        scalar.sqrt(r0[g - 1][:], inv[g - 1][:])._wait_ge(
                            s_inv, g
                        ).then_inc(s_r0)
                for g in (NG - 1,):
                    scalar.sqrt(r0[g][:], inv[g][:])._wait_ge(s_inv, g + 1).then_inc(
                        s_r0
                    )
                # pq half of the output staging
                scalar.copy(
                    stage[0:1, NPAIR * P : OUT_W], psum_pq[:]
                )._wait_ge(s_pe, 1).then_inc(s_stage)

            vd = [0]  # DVE chain-sem value emitted so far

            def _chain(ins):
                ins.then_inc(s_dve)
                vd[0] += 1
                return ins

            def _newton(vector, g):
                # one Newton step r = r0*(1.5 - 0.5*msq*r0^2) -> RR[:, 0:GW],
                # then r^2 -> RR[:, GW:2GW]; all DVE, each step waiting on the
                # previous via the engine chain sem (deep-pipeline RAW hazard)
                _chain(
                    vector.tensor_mul(e[g][:], r0[g][:], r0[g][:])._wait_ge(
                        s_r0, g + 1
                    )
                )
                _chain(
                    vector.tensor_mul(e[g][:], e[g][:], msq[g][:])._wait_ge(
                        s_dve, vd[0]
                    )
                )
                _chain(
                    vector.tensor_scalar(
                        out=cf[g][:], in0=e[g][:], scalar1=-0.5, scalar2=1.5,
                        op0=mybir.AluOpType.mult, op1=mybir.AluOpType.add,
                    )._wait_ge(s_dve, vd[0])
                )
                # RR free layout: (slot, tt, [r_even, r_odd, r2_even, r2_odd])
                RRv = RR[g][:].rearrange("p (s tt f) -> p s tt f", s=GP, tt=4)
                def v4(t):
                    return t[:].rearrange("p (s tt f) -> p s tt f", s=GP, tt=4)
                _chain(
                    vector.tensor_mul(
                        RRv[:, :, :, 0:2], v4(r0[g]), v4(cf[g])
                    )._wait_ge(s_dve, vd[0])
                )
                vector.tensor_mul(
                    RRv[:, :, :, 2:4], RRv[:, :, :, 0:2], RRv[:, :, :, 0:2]
                )._wait_ge(s_dve, vd[0]).then_inc(s_rr)

            @block.vector
            def _(vector):
                vector.memset(stage[:, NPAIR * P : OUT_W], 0.0)
                for g in range(NG):
                    for slot in range(GP):
                        j = g * GP + slot
                        _chain(
                            vector.tensor_reduce(
                                out=msq[g][:, slot * 8 : slot * 8 + 8],
                                in_=xsq[j][:],
                                axis=mybir.AxisListType.X,
                                op=mybir.AluOpType.add,
                            )._wait_ge(s_act, j + 1)
                        )
                    vector.reciprocal(out=inv[g][:], in_=msq[g][:])._wait_ge(
                        s_dve, vd[0]
                    ).then_inc(s_inv)
                    if g >= 1:
                        _newton(vector, g - 1)
                _newton(vector, NG - 1)
                vector.tensor_copy(
                    stage[:, 0 : NPAIR * P], psum_s[:]
                )._wait_ge(s_pe, 1).then_inc(s_stage)

            @block.gpsimd
            def _(gpsimd):
                gpsimd.memset(ones[:], 1.0)
                for g in range(NG):
                    RRv = RR[g][:].rearrange("p (s tt f) -> p s tt f", s=GP, tt=4)
                    r2v = RRv[:, :, :, 2:4]
                    def v4(t, a, b):
                        return t[:, a:b].rearrange(
                            "p (s tt f) -> p s tt f", s=GP, tt=4
                        )
                    gpsimd.tensor_mul(
                        v4(PQ[g], 0, GW), v4(msq[g], 0, GW), r2v
                    )._wait_ge(s_rr, g + 1).then_inc(s_gp)
                    gpsimd.tensor_mul(
                        v4(PQ[g], GW, 2 * GW), v4(PQ[g], 0, GW), r2v
                    )._wait_ge(s_gp, g + 1).then_inc(s_pq)

            @block.tensor
            def _(tensor):
                for g in range(NG):
                    tensor.wait_ge(s_rr, g + 1)
                    for slot in range(GP):
                        j = g * GP + slot
                        tensor.wait_ge(s_dma[j], 16)
                        for tt in range(T // 2):
                            base = slot * 16 + tt * 4
                            mm = tensor.matmul(
                                psum_s[:, P * j : P * (j + 1)],
                                RR[g][:, base : base + 4],
                                xp[j][:, 2 * tt : 2 * tt + 2, :],
                                start=(tt == 0),
                                stop=(tt == T // 2 - 1),
                            )
                    tensor.wait_ge(s_pq, g + 1)
                    mm = tensor.matmul(
                        psum_pq[:, 2 * GW * g : 2 * GW * (g + 1)],
                        ones[:],
                        PQ[g][:].rearrange("p (k s) -> p k s", k=2),
                        start=True,
                        stop=True,
                    )
                    if g == NG - 1:
                        mm.then_inc(s_pe)

        # block exit emitted drain + all-engine barrier; now reset sems so the
        # NEFF can be re-executed
        for s in sems:
            nc.sync.sem_clear(s)

    nc.compile()
    return nc


_NC_CACHE = None


def _get_nc():
    global _NC_CACHE
    if _NC_CACHE is None:
        _NC_CACHE = build_nc()
    return _NC_CACHE


def run_cores(x_full: np.ndarray, trace: bool = False):
    """Shard, run on 8 NeuronCores, return (per-core out blocks, results obj)."""
    nc = _get_nc()
    in_maps = []
    for k in range(NCORES):
        shard = np.ascontiguousarray(
            x_full[:, BPC * k : BPC * (k + 1)].reshape(NPAIR, N, C)
        )
        in_maps.append({"x": shard})
    res = run_bass_kernel_spmd(nc, in_maps, list(range(NCORES)), trace=trace)
    outs = [res.results[k]["out"] for k in range(NCORES)]
    return outs, res


def reduce_host(outs) -> np.ndarray:
    total = 0.0
    for blk in outs:
        blk = blk.astype(np.float64)
        for j in range(NPAIR):
            g, slot = divmod(j, GP)
            s = blk[0, P * j : P * j + 64] + blk[1, P * j + 64 : P * j + 128]
            s2 = blk[2, P * j : P * j + 64] + blk[3, P * j + 64 : P * j + 128]
            pqb = blk[0, NPAIR * P + 2 * GW * g : NPAIR * P + 2 * GW * (g + 1)]
            t_sum = pqb[slot * 8 : slot * 8 + 8].sum()
            t2_sum = pqb[GW + slot * 8 : GW + slot * 8 + 8].sum()
            S0 = np.dot(s, s) - t_sum
            S1 = np.dot(s2, s2) - t2_sum
            total += S0 - EPS * S1
    loss = total / (N * (N - 1)) / B
    return np.array(loss, dtype=np.float32)


def kernel(updated_agents: np.ndarray) -> np.ndarray:
    outs, _ = run_cores(np.asarray(updated_agents))
    return reduce_host(outs)
